# revision 13
# baseline (speedup 1.0000x reference)
"""BatchHardTripletLoss on 8 trn2 NeuronCores (Bass/Tile, SPMD data-parallel).

Strategy: shard anchor rows across cores (512 rows/core). Every core gets the
full transposed embeddings (the "all-gather" is free since the host distributes
full inputs). The pos/neg label masking is folded INTO the Gram matmul via
scaled one-hot label encodings:

    psum[i, j] = e_i . e_j  -  4 * [l_i == l_j]        (e row-normalized)

so for each anchor row i:
    reduce_min(psum[i, :]) = (min sim over positives) - 4   -> hardest positive
    reduce_max(psum[i, :]) =  max sim over negatives        -> hardest negative
(the -4 shift pushes the positive entries strictly below every negative entry:
sims live in [-1, 1]).  per-anchor loss = relu(max - min - 4 + margin) * valid.
Validity (anchor has >=1 other positive and >=1 negative) depends only on
labels and is computed host-side, shipped as a 0/1 mask.

Cross-core reduction: each core returns NM partial sums (one per 128-row
tile); the host adds the 8*NM floats and divides by n_valid.

Implementation notes (trn2 codegen constraints):
  - engine instructions have tiny sync-event budgets (matmul: 1 wait,
    DVE copy/reduce: 1 wait, ACT: 2 waits).  Cross-engine dependency fan-in
    is funneled through tiny "absorber" ops so real instructions stay within
    budget: every PSUM->SBUF copy runs on DVE (so PSUM-ring releases collapse
    into the one DVE semaphore PE already waits on), and PE "touches" every
    DMA-loaded tensor with a 1-element matmul before real use.
  - engine writes at partition offsets must be 32-aligned, so per-chunk
    column-sum results are collected on partition 0 of a [1, B] row and
    reshaped to [NN, 512] by an SBUF->SBUF DMA.
"""

import os
from contextlib import ExitStack

import numpy as np
import ml_dtypes

import concourse.bass as bass
import concourse.bacc as bacc
import concourse.mybir as mybir
import concourse.tile as tile
from concourse.bass_utils import run_bass_kernel_spmd

F32 = mybir.dt.float32
F32R = mybir.dt.float32r
BF16 = mybir.dt.bfloat16
AF = mybir.ActivationFunctionType
ALU = mybir.AluOpType
AX = mybir.AxisListType

B, D, C = 4096, 512, 512
NCORES = 8
RPC = B // NCORES            # rows per core = 512
NCH = 512                    # column chunk size (PSUM bank = 512 fp32)
MARGIN = 0.2
BIG = 4.0

# main-matmul dtype: "f32" (exact, 4 cyc/row) or "f32r" (full rate, ~fp32 acc)
MAIN_DTYPE = os.environ.get("TRIPLET_MAIN_DTYPE", "f32r")


def build_program(Bf=B, Df=D, Cf=C, rpc=RPC, main_dtype=MAIN_DTYPE):
    assert Df % 128 == 0 and Cf % 128 == 0 and Bf % NCH == 0
    assert rpc % 128 == 0 and rpc == NCH, "selector matmul assumes rpc == chunk"
    KD, KC = Df // 128, Cf // 128
    NM = rpc // 128          # 128-row tiles per core
    NN = Bf // NCH           # column chunks
    H = Bf // 2

    nc = bacc.Bacc("TRN2", target_bir_lowering=False, debug=False)
    ET_d = nc.declare_dram_parameter("ET", [Df, Bf], F32, isOutput=False)
    ETo_d = nc.declare_dram_parameter("ETown", [Df, rpc], F32, isOutput=False)
    OTn_d = nc.declare_dram_parameter("OTn", [Cf, Bf], BF16, isOutput=False)
    OTp_d = nc.declare_dram_parameter("OTp", [Cf, rpc], BF16, isOutput=False)
    sel_d = nc.declare_dram_parameter("sel", [NN, 1], F32, isOutput=False)
    val_d = nc.declare_dram_parameter("valid", [128, NM], F32, isOutput=False)
    out_d = nc.declare_dram_parameter("out", [1, NM], F32, isOutput=True)

    with tile.TileContext(nc) as tc, ExitStack() as ctx:
        const = ctx.enter_context(tc.tile_pool(name="const", bufs=1))
        big = ctx.enter_context(tc.tile_pool(name="big", bufs=KD + 1))
        sqp = ctx.enter_context(tc.tile_pool(name="sq", bufs=6))
        otnp = ctx.enter_context(tc.tile_pool(name="otn", bufs=1))
        smalls = ctx.enter_context(tc.tile_pool(name="small", bufs=1))
        psA = ctx.enter_context(tc.tile_pool(name="psA", bufs=2, space="PSUM"))
        psB = ctx.enter_context(tc.tile_pool(name="psB", bufs=2, space="PSUM"))
        psC = ctx.enter_context(tc.tile_pool(name="psC", bufs=1, space="PSUM"))
        psM = ctx.enter_context(tc.tile_pool(name="psM", bufs=3, space="PSUM"))

        def pe_touch(ap, ap2=None):
            """1-element matmul so PE observes a tensor producer's semaphore."""
            t = psC.tile([1, NCH], F32, tag="tiny")
            nc.tensor.matmul(
                t[0:1, 0:1], lhsT=ap, rhs=ap2 if ap2 is not None else ap,
                start=True, stop=True,
            )

        # constants
        ones_cb = const.tile([128, 1], BF16, tag="ones_cb")
        nc.vector.memset(ones_cb[:], 1.0)
        ones_r = const.tile([1, 128], F32, tag="ones_r")
        nc.vector.memset(ones_r[:], 1.0)
        ones_cf = const.tile([128, 1], F32, tag="ones_cf")
        nc.vector.memset(ones_cf[:], 1.0)
        relu_bias = const.tile([128, 1], F32, tag="relu_bias")
        nc.vector.memset(relu_bias[:], MARGIN - BIG)

        sel_t = const.tile([NN, 1], F32, tag="sel")
        nc.sync.dma_start(sel_t[:], sel_d[:, :])
        val_t = const.tile([128, NM], F32, tag="val")
        nc.sync.dma_start(val_t[:], val_d[:, :])

        # ---- load ET (2 half-DMAs per k-tile) --------------------------------
        et_tiles = []
        for k in range(KD):
            t = big.tile([128, Bf], F32, tag="big")
            for j in range(2):
                nc.sync.dma_start(
                    t[:, j * H : (j + 1) * H],
                    ET_d[k * 128 : (k + 1) * 128, j * H : (j + 1) * H],
                )
            et_tiles.append(t)
        eto_tiles = []
        for k in range(KD):
            t = smalls.tile([128, rpc], F32, tag=f"eto{k}")
            nc.sync.dma_start(t[:], ETo_d[k * 128 : (k + 1) * 128, :])
            eto_tiles.append(t)

        # wait-absorbers: ACT and DVE observe the ET DMA queues via tiny ops
        for k in range(KD):
            ab_a = smalls.tile([1, 2], F32, tag=f"abs_a{k}")
            nc.scalar.copy(ab_a[:], et_tiles[k][0:1, H - 1 : H + 1])
            ab_d0 = smalls.tile([1, 1], F32, tag=f"abs_d{k}a")
            nc.vector.tensor_copy(ab_d0[:], et_tiles[k][0:1, 0:1])
            ab_d1 = smalls.tile([1, 1], F32, tag=f"abs_d{k}b")
            nc.vector.tensor_copy(ab_d1[:], et_tiles[k][0:1, H : H + 1])
        ab_v = smalls.tile([1, 1], F32, tag="abs_v")
        nc.vector.tensor_copy(ab_v[:], val_t[0:1, 0:1])

        # ---- column sums of squares -> [1, Bf] row on partition 0 ------------
        row_buf = smalls.tile([1, Bf], F32, tag="rowbuf")
        for j in range(NN):
            if j >= 2:
                # PE observes the DVE copy that released the psA slot (j-2)
                pe_touch(row_buf[0:1, (j - 2) * NCH : (j - 2) * NCH + 1])
            ps = psA.tile([1, NCH], F32, tag="colsum")
            for k in range(KD):
                sq = sqp.tile([128, NCH], BF16, tag="sq")
                nc.scalar.activation(sq[:], et_tiles[k][:, bass.ts(j, NCH)], AF.Square)
                nc.tensor.matmul(
                    ps[:], lhsT=ones_cb[:], rhs=sq[:],
                    start=(k == 0), stop=(k == KD - 1),
                )
            nc.vector.tensor_copy(row_buf[0:1, bass.ts(j, NCH)], ps[:])

        # ---- r = rsqrt(ssq), one Newton polish, on [NN, 512] -----------------
        ssq = smalls.tile([NN, NCH], F32, tag="ssq")
        nc.sync.dma_start(ssq[:, :], row_buf[0:1, :])
        nrm = smalls.tile([NN, NCH], F32, tag="nrm")
        nc.scalar.sqrt(nrm[:], ssq[:])
        r0 = smalls.tile([NN, NCH], F32, tag="r0")
        nc.vector.reciprocal(r0[:], nrm[:])
        t1 = smalls.tile([NN, NCH], F32, tag="nt1")
        nc.vector.tensor_tensor(t1[:], r0[:], r0[:], ALU.mult)
        t2 = smalls.tile([NN, NCH], F32, tag="nt2")
        nc.vector.tensor_tensor(t2[:], t1[:], ssq[:], ALU.mult)
        nc.vector.tensor_scalar(t2[:], t2[:], -0.5, 1.5, ALU.mult, ALU.add)
        r8 = smalls.tile([NN, NCH], F32, tag="r8")
        nc.vector.tensor_tensor(r8[:], r0[:], t2[:], ALU.mult)

        # r as a single [1, Bf] row (partition 0) for the K=1 broadcast matmuls
        r_row = smalls.tile([1, Bf], F32, tag="rrow")
        nc.sync.dma_start(r_row[0:1, :], r8[:, :])

        # PE observes sel/r8/r_row producers
        pe_touch(sel_t[0:1, 0:1])
        pe_touch(r8[0:1, 0:1])
        pe_touch(r_row[0:1, 0:1])

        # own-rows r: r_own = sel.T @ r8  (per-core chunk selection via data)
        rown_ps = psC.tile([1, NCH], F32, tag="tiny")
        nc.tensor.matmul(rown_ps[:], lhsT=sel_t[:], rhs=r8[:, :], start=True, stop=True)
        r_own = smalls.tile([1, NCH], F32, tag="rown")
        nc.vector.tensor_copy(r_own[:], rown_ps[:])

        # ---- broadcast r across partitions (K=1 matmuls), normalize ----------
        rb_sb = []
        for j in range(NN):
            rb_ps = psB.tile([128, NCH], F32, tag="rb")
            nc.tensor.matmul(
                rb_ps[:], lhsT=ones_r[:], rhs=r_row[0:1, bass.ts(j, NCH)],
                start=True, stop=True,
            )
            rbs = smalls.tile([128, NCH], F32, tag=f"rbs{j}")
            nc.vector.tensor_copy(rbs[:], rb_ps[:])
            rb_sb.append(rbs)

        rbo_ps = psB.tile([128, NCH], F32, tag="rb")
        nc.tensor.matmul(rbo_ps[:], lhsT=ones_r[:], rhs=r_own[0:1, :], start=True, stop=True)
        rbo = smalls.tile([128, NCH], F32, tag="rbo")
        nc.vector.tensor_copy(rbo[:], rbo_ps[:])

        eh_tiles = []
        for k in range(KD):
            eh = big.tile([128, Bf], F32, tag="big")
            for j in range(NN):
                nc.vector.tensor_tensor(
                    eh[:, bass.ts(j, NCH)], et_tiles[k][:, bass.ts(j, NCH)],
                    rb_sb[j][:], ALU.mult,
                )
            eh_tiles.append(eh)
        eho_tiles = []
        for k in range(KD):
            eho = smalls.tile([128, rpc], F32, tag=f"eho{k}")
            nc.vector.tensor_tensor(eho[:], eto_tiles[k][:], rbo[:], ALU.mult)
            eho_tiles.append(eho)

        # ---- one-hot label tiles --------------------------------------------
        otn_tiles = []
        for k in range(KC):
            t = otnp.tile([128, Bf], BF16, tag=f"otn{k}")
            for j in range(2):
                nc.sync.dma_start(
                    t[:, j * H : (j + 1) * H],
                    OTn_d[k * 128 : (k + 1) * 128, j * H : (j + 1) * H],
                )
            otn_tiles.append(t)
        otp_tiles = []
        for k in range(KC):
            t = smalls.tile([128, rpc], BF16, tag=f"otp{k}")
            nc.sync.dma_start(t[:], OTp_d[k * 128 : (k + 1) * 128, :])
            otp_tiles.append(t)

        # PE observes the one-hot DMA queues (one matmul per DMA)
        for k in range(KC):
            pe_touch(otn_tiles[k][0:1, 0:1])
            pe_touch(otn_tiles[k][0:1, 0:1], otn_tiles[k][0:1, H : H + 1])
            pe_touch(otp_tiles[k][0:1, 0:1])

        # ---- main fused matmul + masked min/max reductions -------------------
        mm_dt = F32R if main_dtype == "f32r" else F32

        def cast(ap):
            return ap.bitcast(mm_dt) if main_dtype == "f32r" else ap

        loss_all = smalls.tile([128, NM], F32, tag="lossall")
        for m in range(NM):
            mp = smalls.tile([128, NN], F32, tag=f"mp{m}")
            mx = smalls.tile([128, NN], F32, tag=f"mx{m}")
            for n in range(NN):
                ps = psM.tile([128, NCH], F32, tag="ps")
                for k in range(KD):
                    nc.tensor.matmul(
                        ps[:],
                        lhsT=cast(eho_tiles[k][:, bass.ts(m, 128)]),
                        rhs=cast(eh_tiles[k][:, bass.ts(n, NCH)]),
                        start=(k == 0), stop=False,
                    )
                for k in range(KC):
                    nc.tensor.matmul(
                        ps[:],
                        lhsT=otp_tiles[k][:, bass.ts(m, 128)],
                        rhs=otn_tiles[k][:, bass.ts(n, NCH)],
                        start=False, stop=(k == KC - 1),
                    )
                nc.vector.tensor_reduce(mp[:, n : n + 1], ps[:], AX.X, ALU.min)
                nc.vector.tensor_reduce(mx[:, n : n + 1], ps[:], AX.X, ALU.max)
            mpm = smalls.tile([128, 1], F32, tag=f"mpm{m}")
            nc.vector.tensor_reduce(mpm[:], mp[:, :], AX.X, ALU.min)
            mxm = smalls.tile([128, 1], F32, tag=f"mxm{m}")
            nc.vector.tensor_reduce(mxm[:], mx[:, :], AX.X, ALU.max)
            dlt = smalls.tile([128, 1], F32, tag=f"dlt{m}")
            nc.vector.tensor_tensor(dlt[:], mxm[:], mpm[:], ALU.subtract)
            rl = smalls.tile([128, 1], F32, tag=f"rl{m}")
            nc.scalar.activation(rl[:], dlt[:], AF.Relu, bias=relu_bias[:])
            nc.vector.tensor_tensor(
                loss_all[:, m : m + 1], rl[:], val_t[:, m : m + 1], ALU.mult
            )

        # ---- partition-sum of per-anchor losses ------------------------------
        pe_touch(loss_all[0:1, 0:1])
        out_ps = psC.tile([1, NM], F32, tag="tiny")
        nc.tensor.matmul(out_ps[:], lhsT=ones_cf[:], rhs=loss_all[:, :], start=True, stop=True)
        out_sb = smalls.tile([1, NM], F32, tag="outsb")
        nc.vector.tensor_copy(out_sb[:], out_ps[:])
        nc.sync.dma_start(out_d[:, :], out_sb[:])

    nc.compile()
    return nc


def host_prepare(embeddings, labels, Bf=B, Df=D, Cf=C, rpc=RPC):
    """Host-side layout prep + per-core input maps (no embedding math)."""
    embeddings = np.asarray(embeddings, dtype=np.float32)
    labels = np.asarray(labels).astype(np.int64)
    ncores = Bf // rpc
    NM = rpc // 128
    NN = Bf // NCH

    ET = np.ascontiguousarray(embeddings.T)                       # [D, B]
    oh = (np.arange(Cf, dtype=np.int64)[:, None] == labels[None, :])  # [C, B]
    OTn = np.ascontiguousarray((-2.0 * oh).astype(ml_dtypes.bfloat16))
    OTp_full = (2.0 * oh).astype(ml_dtypes.bfloat16)

    cnt = np.bincount(labels, minlength=Cf)[labels]               # class size per anchor
    valid = ((cnt >= 2) & (cnt <= Bf - 1)).astype(np.float32)     # [B]

    in_maps = []
    for c in range(ncores):
        rows = slice(c * rpc, (c + 1) * rpc)
        sel = np.zeros((NN, 1), np.float32)
        sel[c, 0] = 1.0
        in_maps.append(
            {
                "ET": ET,
                "ETown": np.ascontiguousarray(ET[:, rows]),
                "OTn": OTn,
                "OTp": np.ascontiguousarray(OTp_full[:, rows]),
                "sel": sel,
                "valid": np.ascontiguousarray(valid[rows].reshape(NM, 128).T),
            }
        )
    return in_maps, valid


_prog_cache = {}


def _get_program():
    key = (B, D, C, RPC, MAIN_DTYPE)
    if key not in _prog_cache:
        _prog_cache[key] = build_program()
    return _prog_cache[key]


LAST_RESULT = None


def kernel(embeddings, labels):
    global LAST_RESULT
    in_maps, valid = host_prepare(embeddings, labels)
    nc = _get_program()
    trace = bool(int(os.environ.get("TRIPLET_TRACE", "0")))
    res = run_bass_kernel_spmd(nc, in_maps, list(range(NCORES)), trace=trace)
    LAST_RESULT = res
    loss_sum = float(sum(r["out"].astype(np.float64).sum() for r in res.results))
    n_valid = max(int(valid.sum()), 1)
    return np.array(loss_sum / n_valid, dtype=np.float32)


# revision 14
# speedup vs baseline: 1.4301x; 1.4301x over previous
"""BatchHardTripletLoss on 8 trn2 NeuronCores (Bass/Tile, SPMD data-parallel).

Strategy: shard anchor rows across cores (512 rows/core). Every core gets the
full transposed embeddings (the "all-gather" is free since the host distributes
full inputs). The pos/neg label masking is folded INTO the Gram matmul via
scaled one-hot label encodings:

    psum[i, j] = e_i . e_j  -  4 * [l_i == l_j]        (e row-normalized)

so for each anchor row i:
    reduce_min(psum[i, :]) = (min sim over positives) - 4   -> hardest positive
    reduce_max(psum[i, :]) =  max sim over negatives        -> hardest negative
(the -4 shift pushes the positive entries strictly below every negative entry:
sims live in [-1, 1]).  per-anchor loss = relu(max - min - 4 + margin) * valid.
Validity (anchor has >=1 other positive and >=1 negative) depends only on
labels and is computed host-side, shipped as a 0/1 mask.

Cross-core reduction: each core returns NM partial sums (one per 128-row
tile); the host adds the 8*NM floats and divides by n_valid.

Implementation notes (trn2 codegen constraints):
  - engine instructions have tiny sync-event budgets (matmul: 1 wait,
    DVE copy/reduce: 1 wait, ACT: 2 waits).  Cross-engine dependency fan-in
    is funneled through tiny "absorber" ops so real instructions stay within
    budget: every PSUM->SBUF copy runs on DVE (so PSUM-ring releases collapse
    into the one DVE semaphore PE already waits on), and PE "touches" every
    DMA-loaded tensor with a 1-element matmul before real use.
  - engine writes at partition offsets must be 32-aligned, so per-chunk
    column-sum results are collected on partition 0 of a [1, B] row and
    reshaped to [NN, 512] by an SBUF->SBUF DMA.
"""

import os
from contextlib import ExitStack

import numpy as np
import ml_dtypes

import concourse.bass as bass
import concourse.bacc as bacc
import concourse.mybir as mybir
import concourse.tile as tile
from concourse.bass_utils import run_bass_kernel_spmd

F32 = mybir.dt.float32
F32R = mybir.dt.float32r
BF16 = mybir.dt.bfloat16
AF = mybir.ActivationFunctionType
ALU = mybir.AluOpType
AX = mybir.AxisListType

B, D, C = 4096, 512, 512
NCORES = 8
RPC = B // NCORES            # rows per core = 512
NCH = 512                    # column chunk size (PSUM bank = 512 fp32)
MARGIN = 0.2
BIG = 4.0

# main-matmul dtype: "f32" (exact, 4 cyc/row) or "f32r" (full rate, ~fp32 acc)
MAIN_DTYPE = os.environ.get("TRIPLET_MAIN_DTYPE", "f32r")


def build_program(Bf=B, Df=D, Cf=C, rpc=RPC, main_dtype=MAIN_DTYPE):
    assert Df % 128 == 0 and Cf % 128 == 0 and Bf % NCH == 0
    assert rpc % 128 == 0 and rpc == NCH, "selector matmul assumes rpc == chunk"
    KD, KC = Df // 128, Cf // 128
    NM = rpc // 128          # 128-row tiles per core
    NN = Bf // NCH           # column chunks
    H = Bf // 2

    nc = bacc.Bacc("TRN2", target_bir_lowering=False, debug=False)
    ET_d = nc.declare_dram_parameter("ET", [Df, Bf], F32, isOutput=False)
    ETo_d = nc.declare_dram_parameter("ETown", [Df, rpc], F32, isOutput=False)
    OTn_d = nc.declare_dram_parameter("OTn", [Cf, Bf], BF16, isOutput=False)
    OTp_d = nc.declare_dram_parameter("OTp", [Cf, rpc], BF16, isOutput=False)
    sel_d = nc.declare_dram_parameter("sel", [NN, 1], F32, isOutput=False)
    val_d = nc.declare_dram_parameter("valid", [128, NM], F32, isOutput=False)
    out_d = nc.declare_dram_parameter("out", [1, NM], F32, isOutput=True)

    with tile.TileContext(nc) as tc, ExitStack() as ctx:
        const = ctx.enter_context(tc.tile_pool(name="const", bufs=1))
        big = ctx.enter_context(tc.tile_pool(name="big", bufs=KD + 1))
        sqp = ctx.enter_context(tc.tile_pool(name="sq", bufs=6))
        otnp = ctx.enter_context(tc.tile_pool(name="otn", bufs=1))
        smalls = ctx.enter_context(tc.tile_pool(name="small", bufs=1))
        psA = ctx.enter_context(tc.tile_pool(name="psA", bufs=2, space="PSUM"))
        psB = ctx.enter_context(tc.tile_pool(name="psB", bufs=2, space="PSUM"))
        psC = ctx.enter_context(tc.tile_pool(name="psC", bufs=1, space="PSUM"))
        psM = ctx.enter_context(tc.tile_pool(name="psM", bufs=3, space="PSUM"))

        def pe_touch(ap, ap2=None):
            """1-element matmul so PE observes a tensor producer's semaphore."""
            t = psC.tile([1, NCH], F32, tag="tiny")
            nc.tensor.matmul(
                t[0:1, 0:1], lhsT=ap, rhs=ap2 if ap2 is not None else ap,
                start=True, stop=True,
            )

        # constants
        ones_cb = const.tile([128, 1], BF16, tag="ones_cb")
        nc.vector.memset(ones_cb[:], 1.0)
        ones_r = const.tile([1, 128], F32, tag="ones_r")
        nc.vector.memset(ones_r[:], 1.0)
        ones_cf = const.tile([128, 1], F32, tag="ones_cf")
        nc.vector.memset(ones_cf[:], 1.0)
        relu_bias = const.tile([128, 1], F32, tag="relu_bias")
        nc.vector.memset(relu_bias[:], MARGIN - BIG)

        sel_t = const.tile([NN, 1], F32, tag="sel")
        nc.sync.dma_start(sel_t[:], sel_d[:, :])
        val_t = const.tile([128, NM], F32, tag="val")
        nc.sync.dma_start(val_t[:], val_d[:, :])

        # ---- load ET (2 half-DMAs per k-tile) --------------------------------
        et_tiles = []
        for k in range(KD):
            t = big.tile([128, Bf], F32, tag="big")
            for j in range(2):
                nc.sync.dma_start(
                    t[:, j * H : (j + 1) * H],
                    ET_d[k * 128 : (k + 1) * 128, j * H : (j + 1) * H],
                )
            et_tiles.append(t)
        eto_tiles = []
        for k in range(KD):
            t = smalls.tile([128, rpc], F32, tag=f"eto{k}")
            nc.sync.dma_start(t[:], ETo_d[k * 128 : (k + 1) * 128, :])
            eto_tiles.append(t)

        # wait-absorbers: ACT and DVE observe the ET DMA queues via tiny ops
        for k in range(KD):
            ab_a = smalls.tile([1, 2], F32, tag=f"abs_a{k}")
            nc.scalar.copy(ab_a[:], et_tiles[k][0:1, H - 1 : H + 1])
            ab_d0 = smalls.tile([1, 1], F32, tag=f"abs_d{k}a")
            nc.vector.tensor_copy(ab_d0[:], et_tiles[k][0:1, 0:1])
            ab_d1 = smalls.tile([1, 1], F32, tag=f"abs_d{k}b")
            nc.vector.tensor_copy(ab_d1[:], et_tiles[k][0:1, H : H + 1])
        ab_v = smalls.tile([1, 1], F32, tag="abs_v")
        nc.vector.tensor_copy(ab_v[:], val_t[0:1, 0:1])

        # ---- column sums of squares -> [1, Bf] row on partition 0 ------------
        row_buf = smalls.tile([1, Bf], F32, tag="rowbuf")
        for j in range(NN):
            if j >= 2:
                # PE observes the DVE copy that released the psA slot (j-2)
                pe_touch(row_buf[0:1, (j - 2) * NCH : (j - 2) * NCH + 1])
            ps = psA.tile([1, NCH], F32, tag="colsum")
            for k in range(KD):
                sq = sqp.tile([128, NCH], BF16, tag="sq")
                nc.scalar.activation(sq[:], et_tiles[k][:, bass.ts(j, NCH)], AF.Square)
                nc.tensor.matmul(
                    ps[:], lhsT=ones_cb[:], rhs=sq[:],
                    start=(k == 0), stop=(k == KD - 1),
                )
            nc.vector.tensor_copy(row_buf[0:1, bass.ts(j, NCH)], ps[:])

        # ---- r = rsqrt(ssq), one Newton polish, on [NN, 512] -----------------
        ssq = smalls.tile([NN, NCH], F32, tag="ssq")
        nc.sync.dma_start(ssq[:, :], row_buf[0:1, :])
        nrm = smalls.tile([NN, NCH], F32, tag="nrm")
        nc.scalar.sqrt(nrm[:], ssq[:])
        r0 = smalls.tile([NN, NCH], F32, tag="r0")
        nc.vector.reciprocal(r0[:], nrm[:])
        t1 = smalls.tile([NN, NCH], F32, tag="nt1")
        nc.vector.tensor_tensor(t1[:], r0[:], r0[:], ALU.mult)
        t2 = smalls.tile([NN, NCH], F32, tag="nt2")
        nc.vector.tensor_tensor(t2[:], t1[:], ssq[:], ALU.mult)
        nc.vector.tensor_scalar(t2[:], t2[:], -0.5, 1.5, ALU.mult, ALU.add)
        r8 = smalls.tile([NN, NCH], F32, tag="r8")
        nc.vector.tensor_tensor(r8[:], r0[:], t2[:], ALU.mult)

        # r as a single [1, Bf] row (partition 0) for the K=1 broadcast matmuls
        r_row = smalls.tile([1, Bf], F32, tag="rrow")
        nc.sync.dma_start(r_row[0:1, :], r8[:, :])

        # PE observes sel/r8/r_row producers
        pe_touch(sel_t[0:1, 0:1])
        pe_touch(r8[0:1, 0:1])
        pe_touch(r_row[0:1, 0:1])

        # own-rows r: r_own = sel.T @ r8  (per-core chunk selection via data)
        rown_ps = psC.tile([1, NCH], F32, tag="tiny")
        nc.tensor.matmul(rown_ps[:], lhsT=sel_t[:], rhs=r8[:, :], start=True, stop=True)
        r_own = smalls.tile([1, NCH], F32, tag="rown")
        nc.vector.tensor_copy(r_own[:], rown_ps[:])

        # ---- broadcast r across partitions (K=1 matmuls), normalize ----------
        rb_sb = []
        for j in range(NN):
            rb_ps = psB.tile([128, NCH], F32, tag="rb")
            nc.tensor.matmul(
                rb_ps[:], lhsT=ones_r[:], rhs=r_row[0:1, bass.ts(j, NCH)],
                start=True, stop=True,
            )
            rbs = smalls.tile([128, NCH], F32, tag=f"rbs{j}")
            nc.vector.tensor_copy(rbs[:], rb_ps[:])
            rb_sb.append(rbs)

        rbo_ps = psB.tile([128, NCH], F32, tag="rb")
        nc.tensor.matmul(rbo_ps[:], lhsT=ones_r[:], rhs=r_own[0:1, :], start=True, stop=True)
        rbo = smalls.tile([128, NCH], F32, tag="rbo")
        nc.vector.tensor_copy(rbo[:], rbo_ps[:])

        mm_dt = F32R if main_dtype == "f32r" else F32
        eh_tiles = []
        for k in range(KD):
            eh = big.tile([128, Bf], mm_dt, tag="big")
            for j in range(NN):
                nc.vector.tensor_tensor(
                    eh[:, bass.ts(j, NCH)], et_tiles[k][:, bass.ts(j, NCH)],
                    rb_sb[j][:], ALU.mult,
                )
            eh_tiles.append(eh)
        eho_tiles = []
        for k in range(KD):
            eho = smalls.tile([128, rpc], mm_dt, tag=f"eho{k}")
            nc.vector.tensor_tensor(eho[:], eto_tiles[k][:], rbo[:], ALU.mult)
            eho_tiles.append(eho)

        # ---- one-hot label tiles --------------------------------------------
        otn_tiles = []
        for k in range(KC):
            t = otnp.tile([128, Bf], BF16, tag=f"otn{k}")
            for j in range(2):
                nc.sync.dma_start(
                    t[:, j * H : (j + 1) * H],
                    OTn_d[k * 128 : (k + 1) * 128, j * H : (j + 1) * H],
                )
            otn_tiles.append(t)
        otp_tiles = []
        for k in range(KC):
            t = smalls.tile([128, rpc], BF16, tag=f"otp{k}")
            nc.sync.dma_start(t[:], OTp_d[k * 128 : (k + 1) * 128, :])
            otp_tiles.append(t)

        # PE observes the one-hot DMA queues (one matmul per DMA)
        for k in range(KC):
            pe_touch(otn_tiles[k][0:1, 0:1])
            pe_touch(otn_tiles[k][0:1, 0:1], otn_tiles[k][0:1, H : H + 1])
            pe_touch(otp_tiles[k][0:1, 0:1])

        # ---- main fused matmul + masked min/max reductions -------------------
        loss_all = smalls.tile([128, NM], F32, tag="lossall")
        for m in range(NM):
            mp = smalls.tile([128, NN], F32, tag=f"mp{m}")
            mx = smalls.tile([128, NN], F32, tag=f"mx{m}")
            for n in range(NN):
                ps = psM.tile([128, NCH], F32, tag="ps")
                for k in range(KD):
                    nc.tensor.matmul(
                        ps[:],
                        lhsT=eho_tiles[k][:, bass.ts(m, 128)],
                        rhs=eh_tiles[k][:, bass.ts(n, NCH)],
                        start=(k == 0), stop=False,
                    )
                for k in range(KC):
                    nc.tensor.matmul(
                        ps[:],
                        lhsT=otp_tiles[k][:, bass.ts(m, 128)],
                        rhs=otn_tiles[k][:, bass.ts(n, NCH)],
                        start=False, stop=(k == KC - 1),
                    )
                nc.vector.tensor_reduce(mp[:, n : n + 1], ps[:], AX.X, ALU.min)
                nc.vector.tensor_reduce(mx[:, n : n + 1], ps[:], AX.X, ALU.max)
            mpm = smalls.tile([128, 1], F32, tag=f"mpm{m}")
            nc.vector.tensor_reduce(mpm[:], mp[:, :], AX.X, ALU.min)
            mxm = smalls.tile([128, 1], F32, tag=f"mxm{m}")
            nc.vector.tensor_reduce(mxm[:], mx[:, :], AX.X, ALU.max)
            dlt = smalls.tile([128, 1], F32, tag=f"dlt{m}")
            nc.vector.tensor_tensor(dlt[:], mxm[:], mpm[:], ALU.subtract)
            rl = smalls.tile([128, 1], F32, tag=f"rl{m}")
            nc.scalar.activation(rl[:], dlt[:], AF.Relu, bias=relu_bias[:])
            nc.vector.tensor_tensor(
                loss_all[:, m : m + 1], rl[:], val_t[:, m : m + 1], ALU.mult
            )

        # ---- partition-sum of per-anchor losses ------------------------------
        pe_touch(loss_all[0:1, 0:1])
        out_ps = psC.tile([1, NM], F32, tag="tiny")
        nc.tensor.matmul(out_ps[:], lhsT=ones_cf[:], rhs=loss_all[:, :], start=True, stop=True)
        out_sb = smalls.tile([1, NM], F32, tag="outsb")
        nc.vector.tensor_copy(out_sb[:], out_ps[:])
        nc.sync.dma_start(out_d[:, :], out_sb[:])

    nc.compile()
    return nc


def host_prepare(embeddings, labels, Bf=B, Df=D, Cf=C, rpc=RPC):
    """Host-side layout prep + per-core input maps (no embedding math)."""
    embeddings = np.asarray(embeddings, dtype=np.float32)
    labels = np.asarray(labels).astype(np.int64)
    ncores = Bf // rpc
    NM = rpc // 128
    NN = Bf // NCH

    ET = np.ascontiguousarray(embeddings.T)                       # [D, B]
    oh = (np.arange(Cf, dtype=np.int64)[:, None] == labels[None, :])  # [C, B]
    OTn = np.ascontiguousarray((-2.0 * oh).astype(ml_dtypes.bfloat16))
    OTp_full = (2.0 * oh).astype(ml_dtypes.bfloat16)

    cnt = np.bincount(labels, minlength=Cf)[labels]               # class size per anchor
    valid = ((cnt >= 2) & (cnt <= Bf - 1)).astype(np.float32)     # [B]

    in_maps = []
    for c in range(ncores):
        rows = slice(c * rpc, (c + 1) * rpc)
        sel = np.zeros((NN, 1), np.float32)
        sel[c, 0] = 1.0
        in_maps.append(
            {
                "ET": ET,
                "ETown": np.ascontiguousarray(ET[:, rows]),
                "OTn": OTn,
                "OTp": np.ascontiguousarray(OTp_full[:, rows]),
                "sel": sel,
                "valid": np.ascontiguousarray(valid[rows].reshape(NM, 128).T),
            }
        )
    return in_maps, valid


_prog_cache = {}


def _get_program():
    key = (B, D, C, RPC, MAIN_DTYPE)
    if key not in _prog_cache:
        _prog_cache[key] = build_program()
    return _prog_cache[key]


LAST_RESULT = None


def kernel(embeddings, labels):
    global LAST_RESULT
    in_maps, valid = host_prepare(embeddings, labels)
    nc = _get_program()
    trace = bool(int(os.environ.get("TRIPLET_TRACE", "0")))
    res = run_bass_kernel_spmd(nc, in_maps, list(range(NCORES)), trace=trace)
    LAST_RESULT = res
    loss_sum = float(sum(r["out"].astype(np.float64).sum() for r in res.results))
    n_valid = max(int(valid.sum()), 1)
    return np.array(loss_sum / n_valid, dtype=np.float32)


# revision 17
# speedup vs baseline: 1.5404x; 1.0772x over previous
"""BatchHardTripletLoss on 8 trn2 NeuronCores (Bass/Tile, SPMD data-parallel).

Strategy: shard anchor rows across cores (512 rows/core). Every core gets the
full transposed embeddings (the "all-gather" is free since the host distributes
full inputs). The pos/neg label masking is folded INTO the Gram matmul via
scaled one-hot label encodings:

    psum[i, j] = e_i . e_j  -  4 * [l_i == l_j]        (e row-normalized)

so for each anchor row i:
    reduce_min(psum[i, :]) = (min sim over positives) - 4   -> hardest positive
    reduce_max(psum[i, :]) =  max sim over negatives        -> hardest negative
(the -4 shift pushes the positive entries strictly below every negative entry:
sims live in [-1, 1]).  per-anchor loss = relu(max - min - 4 + margin) * valid.
Validity (anchor has >=1 other positive and >=1 negative) depends only on
labels and is computed host-side, shipped as a 0/1 mask.

Cross-core reduction: each core returns NM partial sums (one per 128-row
tile); the host adds the 8*NM floats and divides by n_valid.

Implementation notes (trn2 codegen constraints):
  - engine instructions have tiny sync-event budgets (matmul: 1 wait,
    DVE copy/reduce: 1 wait, ACT: 2 waits).  Cross-engine dependency fan-in
    is funneled through tiny "absorber" ops so real instructions stay within
    budget: every PSUM->SBUF copy runs on DVE (so PSUM-ring releases collapse
    into the one DVE semaphore PE already waits on), and PE "touches" every
    DMA-loaded tensor with a 1-element matmul before real use.
  - engine writes at partition offsets must be 32-aligned, so per-chunk
    column-sum results are collected on partition 0 of a [1, B] row and
    reshaped to [NN, 512] by an SBUF->SBUF DMA.
"""

import os
from contextlib import ExitStack

import numpy as np
import ml_dtypes

import concourse.bass as bass
import concourse.bacc as bacc
import concourse.mybir as mybir
import concourse.tile as tile
from concourse.bass_utils import run_bass_kernel_spmd

F32 = mybir.dt.float32
F32R = mybir.dt.float32r
BF16 = mybir.dt.bfloat16
AF = mybir.ActivationFunctionType
ALU = mybir.AluOpType
AX = mybir.AxisListType

B, D, C = 4096, 512, 512
NCORES = 8
RPC = B // NCORES            # rows per core = 512
NCH = 512                    # column chunk size (PSUM bank = 512 fp32)
MARGIN = 0.2
BIG = 4.0

# main-matmul dtype: "f32" (exact, 4 cyc/row) or "f32r" (full rate, ~fp32 acc)
MAIN_DTYPE = os.environ.get("TRIPLET_MAIN_DTYPE", "f32r")


def build_program(Bf=B, Df=D, Cf=C, rpc=RPC, main_dtype=MAIN_DTYPE):
    assert Df % 128 == 0 and Cf % 128 == 0 and Bf % NCH == 0
    assert rpc % 128 == 0 and rpc == NCH, "selector matmul assumes rpc == chunk"
    KD, KC = Df // 128, Cf // 128
    NM = rpc // 128          # 128-row tiles per core
    NN = Bf // NCH           # column chunks
    H = Bf // 2

    mm_dt = F32R if main_dtype == "f32r" else F32
    nc = bacc.Bacc("TRN2", target_bir_lowering=False, debug=False)
    ET_d = nc.declare_dram_parameter("ET", [Df, Bf], mm_dt, isOutput=False)
    ETo_d = nc.declare_dram_parameter("ETown", [Df, rpc], mm_dt, isOutput=False)
    OTn_d = nc.declare_dram_parameter("OTn", [Cf, Bf], BF16, isOutput=False)
    OTp_d = nc.declare_dram_parameter("OTp", [Cf, rpc], BF16, isOutput=False)
    sel_d = nc.declare_dram_parameter("sel", [NN, 1], F32, isOutput=False)
    val_d = nc.declare_dram_parameter("valid", [128, NM], F32, isOutput=False)
    out_d = nc.declare_dram_parameter("out", [1, NM], F32, isOutput=True)

    with tile.TileContext(nc) as tc, ExitStack() as ctx:
        const = ctx.enter_context(tc.tile_pool(name="const", bufs=1))
        big = ctx.enter_context(tc.tile_pool(name="big", bufs=KD))
        sqp = ctx.enter_context(tc.tile_pool(name="sq", bufs=6))
        otnp = ctx.enter_context(tc.tile_pool(name="otn", bufs=1))
        smalls = ctx.enter_context(tc.tile_pool(name="small", bufs=1))
        psA = ctx.enter_context(tc.tile_pool(name="psA", bufs=2, space="PSUM"))
        psB = ctx.enter_context(tc.tile_pool(name="psB", bufs=2, space="PSUM"))
        psC = ctx.enter_context(tc.tile_pool(name="psC", bufs=1, space="PSUM"))
        psM = ctx.enter_context(tc.tile_pool(name="psM", bufs=3, space="PSUM"))

        def pe_touch(ap, ap2=None):
            """1-element matmul so PE observes a tensor producer's semaphore."""
            t = psC.tile([1, NCH], F32, tag="tiny")
            nc.tensor.matmul(
                t[0:1, 0:1], lhsT=ap, rhs=ap2 if ap2 is not None else ap,
                start=True, stop=True,
            )

        # constants
        ones_cb = const.tile([128, 1], BF16, tag="ones_cb")
        nc.vector.memset(ones_cb[:], 1.0)
        ones_r = const.tile([1, 128], F32, tag="ones_r")
        nc.vector.memset(ones_r[:], 1.0)
        ones_cf = const.tile([128, 1], F32, tag="ones_cf")
        nc.vector.memset(ones_cf[:], 1.0)
        relu_bias = const.tile([128, 1], F32, tag="relu_bias")
        nc.vector.memset(relu_bias[:], MARGIN - BIG)

        sel_t = const.tile([NN, 1], F32, tag="sel")
        nc.sync.dma_start(sel_t[:], sel_d[:, :])
        val_t = const.tile([128, NM], F32, tag="val")
        nc.sync.dma_start(val_t[:], val_d[:, :])

        # ---- load ET (2 half-DMAs per k-tile) --------------------------------
        et_tiles = []
        for k in range(KD):
            t = big.tile([128, Bf], mm_dt, tag="big")
            for j in range(2):
                nc.sync.dma_start(
                    t[:, j * H : (j + 1) * H],
                    ET_d[k * 128 : (k + 1) * 128, j * H : (j + 1) * H],
                )
            et_tiles.append(t)
        eto_tiles = []
        for k in range(KD):
            t = smalls.tile([128, rpc], mm_dt, tag=f"eto{k}")
            nc.sync.dma_start(t[:], ETo_d[k * 128 : (k + 1) * 128, :])
            eto_tiles.append(t)

        # wait-absorbers: ACT and DVE observe the ET DMA queues via tiny ops
        for k in range(KD):
            ab_a = smalls.tile([1, 2], F32, tag=f"abs_a{k}")
            nc.scalar.copy(ab_a[:], et_tiles[k][0:1, H - 1 : H + 1])
            ab_d0 = smalls.tile([1, 1], F32, tag=f"abs_d{k}a")
            nc.vector.tensor_copy(ab_d0[:], et_tiles[k][0:1, 0:1])
            ab_d1 = smalls.tile([1, 1], F32, tag=f"abs_d{k}b")
            nc.vector.tensor_copy(ab_d1[:], et_tiles[k][0:1, H : H + 1])
        ab_v = smalls.tile([1, 1], F32, tag="abs_v")
        nc.vector.tensor_copy(ab_v[:], val_t[0:1, 0:1])

        # ---- column sums of squares -> [1, Bf] row on partition 0 ------------
        row_buf = smalls.tile([1, Bf], F32, tag="rowbuf")
        for j in range(NN):
            if j >= 2:
                # PE observes the DVE copy that released the psA slot (j-2)
                pe_touch(row_buf[0:1, (j - 2) * NCH : (j - 2) * NCH + 1])
            ps = psA.tile([1, NCH], F32, tag="colsum")
            for k in range(KD):
                sq = sqp.tile([128, NCH], BF16, tag="sq")
                nc.scalar.activation(sq[:], et_tiles[k][:, bass.ts(j, NCH)], AF.Square)
                nc.tensor.matmul(
                    ps[:], lhsT=ones_cb[:], rhs=sq[:],
                    start=(k == 0), stop=(k == KD - 1),
                )
            nc.vector.tensor_copy(row_buf[0:1, bass.ts(j, NCH)], ps[:])

        # ---- r = rsqrt(ssq), one Newton polish, on [NN, 512] -----------------
        ssq = smalls.tile([NN, NCH], F32, tag="ssq")
        nc.sync.dma_start(ssq[:, :], row_buf[0:1, :])
        nrm = smalls.tile([NN, NCH], F32, tag="nrm")
        nc.scalar.sqrt(nrm[:], ssq[:])
        r0 = smalls.tile([NN, NCH], F32, tag="r0")
        nc.vector.reciprocal(r0[:], nrm[:])
        t1 = smalls.tile([NN, NCH], F32, tag="nt1")
        nc.vector.tensor_tensor(t1[:], r0[:], r0[:], ALU.mult)
        t2 = smalls.tile([NN, NCH], F32, tag="nt2")
        nc.vector.tensor_tensor(t2[:], t1[:], ssq[:], ALU.mult)
        nc.vector.tensor_scalar(t2[:], t2[:], -0.5, 1.5, ALU.mult, ALU.add)
        r8 = smalls.tile([NN, NCH], F32, tag="r8")
        nc.vector.tensor_tensor(r8[:], r0[:], t2[:], ALU.mult)

        # r as a single [1, Bf] row (partition 0) for the K=1 broadcast matmuls
        r_row = smalls.tile([1, Bf], F32, tag="rrow")
        nc.sync.dma_start(r_row[0:1, :], r8[:, :])

        # PE observes sel/r8/r_row producers
        pe_touch(sel_t[0:1, 0:1])
        pe_touch(r8[0:1, 0:1])
        pe_touch(r_row[0:1, 0:1])

        # own-rows r: r_own = sel.T @ r8  (per-core chunk selection via data)
        rown_ps = psC.tile([1, NCH], F32, tag="tiny")
        nc.tensor.matmul(rown_ps[:], lhsT=sel_t[:], rhs=r8[:, :], start=True, stop=True)
        r_own = smalls.tile([1, NCH], F32, tag="rown")
        nc.vector.tensor_copy(r_own[:], rown_ps[:])

        # ---- broadcast r across partitions (K=1 matmuls); normalize in place -
        # (j-outer so the first column chunks of e-hat are ready early and the
        # main matmul overlaps the rest of the normalization)
        rbo_ps = psB.tile([128, NCH], F32, tag="rb")
        nc.tensor.matmul(rbo_ps[:], lhsT=ones_r[:], rhs=r_own[0:1, :], start=True, stop=True)
        eho_tiles = []
        for k in range(KD):
            eho = eto_tiles[k]
            nc.vector.tensor_tensor(eho[:], eto_tiles[k][:], rbo_ps[:], ALU.mult)
            eho_tiles.append(eho)
        eh_tiles = et_tiles
        for j in range(NN):
            rb_ps = psB.tile([128, NCH], F32, tag="rb")
            nc.tensor.matmul(
                rb_ps[:], lhsT=ones_r[:], rhs=r_row[0:1, bass.ts(j, NCH)],
                start=True, stop=True,
            )
            for k in range(KD):
                nc.vector.tensor_tensor(
                    eh_tiles[k][:, bass.ts(j, NCH)], et_tiles[k][:, bass.ts(j, NCH)],
                    rb_ps[:], ALU.mult,
                )

        # ---- one-hot label tiles --------------------------------------------
        otn_tiles = []
        for k in range(KC):
            t = otnp.tile([128, Bf], BF16, tag=f"otn{k}")
            for j in range(2):
                nc.sync.dma_start(
                    t[:, j * H : (j + 1) * H],
                    OTn_d[k * 128 : (k + 1) * 128, j * H : (j + 1) * H],
                )
            otn_tiles.append(t)
        otp_tiles = []
        for k in range(KC):
            t = smalls.tile([128, rpc], BF16, tag=f"otp{k}")
            nc.sync.dma_start(t[:], OTp_d[k * 128 : (k + 1) * 128, :])
            otp_tiles.append(t)

        # PE observes the one-hot DMA queues (one matmul per DMA)
        for k in range(KC):
            pe_touch(otn_tiles[k][0:1, 0:1])
            pe_touch(otn_tiles[k][0:1, 0:1], otn_tiles[k][0:1, H : H + 1])
            pe_touch(otp_tiles[k][0:1, 0:1])

        # ---- main fused matmul + masked min/max reductions -------------------
        loss_all = smalls.tile([128, NM], F32, tag="lossall")
        mps = [smalls.tile([128, NN], F32, tag=f"mp{m}", name=f"mp{m}") for m in range(NM)]
        mxs = [smalls.tile([128, NN], F32, tag=f"mx{m}", name=f"mx{m}") for m in range(NM)]
        for n in range(NN):
            for m in range(NM):
                ps = psM.tile([128, NCH], F32, tag="ps")
                for k in range(KC):
                    nc.tensor.matmul(
                        ps[:],
                        lhsT=otp_tiles[k][:, bass.ts(m, 128)],
                        rhs=otn_tiles[k][:, bass.ts(n, NCH)],
                        start=(k == 0), stop=False,
                    )
                for k in range(KD):
                    nc.tensor.matmul(
                        ps[:],
                        lhsT=eho_tiles[k][:, bass.ts(m, 128)],
                        rhs=eh_tiles[k][:, bass.ts(n, NCH)],
                        start=False, stop=(k == KD - 1),
                    )
                nc.vector.tensor_reduce(mps[m][:, n : n + 1], ps[:], AX.X, ALU.min)
                nc.vector.tensor_reduce(mxs[m][:, n : n + 1], ps[:], AX.X, ALU.max)
        for m in range(NM):
            mpm = smalls.tile([128, 1], F32, tag=f"mpm{m}")
            nc.vector.tensor_reduce(mpm[:], mps[m][:, :], AX.X, ALU.min)
            mxm = smalls.tile([128, 1], F32, tag=f"mxm{m}")
            nc.vector.tensor_reduce(mxm[:], mxs[m][:, :], AX.X, ALU.max)
            dlt = smalls.tile([128, 1], F32, tag=f"dlt{m}")
            nc.vector.tensor_tensor(dlt[:], mxm[:], mpm[:], ALU.subtract)
            rl = smalls.tile([128, 1], F32, tag=f"rl{m}")
            nc.scalar.activation(rl[:], dlt[:], AF.Relu, bias=relu_bias[:])
            nc.vector.tensor_tensor(
                loss_all[:, m : m + 1], rl[:], val_t[:, m : m + 1], ALU.mult
            )

        # ---- partition-sum of per-anchor losses ------------------------------
        pe_touch(loss_all[0:1, 0:1])
        out_ps = psC.tile([1, NM], F32, tag="tiny")
        nc.tensor.matmul(out_ps[:], lhsT=ones_cf[:], rhs=loss_all[:, :], start=True, stop=True)
        out_sb = smalls.tile([1, NM], F32, tag="outsb")
        nc.vector.tensor_copy(out_sb[:], out_ps[:])
        nc.sync.dma_start(out_d[:, :], out_sb[:])

    nc.compile()
    return nc


def host_prepare(embeddings, labels, Bf=B, Df=D, Cf=C, rpc=RPC):
    """Host-side layout prep + per-core input maps (no embedding math)."""
    embeddings = np.asarray(embeddings, dtype=np.float32)
    labels = np.asarray(labels).astype(np.int64)
    ncores = Bf // rpc
    NM = rpc // 128
    NN = Bf // NCH

    ET = np.ascontiguousarray(embeddings.T)                       # [D, B]
    oh = (np.arange(Cf, dtype=np.int64)[:, None] == labels[None, :])  # [C, B]
    OTn = np.ascontiguousarray((-2.0 * oh).astype(ml_dtypes.bfloat16))
    OTp_full = (2.0 * oh).astype(ml_dtypes.bfloat16)

    cnt = np.bincount(labels, minlength=Cf)[labels]               # class size per anchor
    valid = ((cnt >= 2) & (cnt <= Bf - 1)).astype(np.float32)     # [B]

    in_maps = []
    for c in range(ncores):
        rows = slice(c * rpc, (c + 1) * rpc)
        sel = np.zeros((NN, 1), np.float32)
        sel[c, 0] = 1.0
        in_maps.append(
            {
                "ET": ET,
                "ETown": np.ascontiguousarray(ET[:, rows]),
                "OTn": OTn,
                "OTp": np.ascontiguousarray(OTp_full[:, rows]),
                "sel": sel,
                "valid": np.ascontiguousarray(valid[rows].reshape(NM, 128).T),
            }
        )
    return in_maps, valid


_prog_cache = {}


def _get_program():
    key = (B, D, C, RPC, MAIN_DTYPE)
    if key not in _prog_cache:
        _prog_cache[key] = build_program()
    return _prog_cache[key]


LAST_RESULT = None


def kernel(embeddings, labels):
    global LAST_RESULT
    in_maps, valid = host_prepare(embeddings, labels)
    nc = _get_program()
    trace = bool(int(os.environ.get("TRIPLET_TRACE", "0")))
    res = run_bass_kernel_spmd(nc, in_maps, list(range(NCORES)), trace=trace)
    LAST_RESULT = res
    loss_sum = float(sum(r["out"].astype(np.float64).sum() for r in res.results))
    n_valid = max(int(valid.sum()), 1)
    return np.array(loss_sum / n_valid, dtype=np.float32)
